# revision 1
# baseline (speedup 1.0000x reference)
"""EHM (SMPLX body + FLAME head + MANO hands) Bass kernel for 8 TRN2 NeuronCores.

Sharding: VERTEX sharding — model weights (shapedirs/posedirs/regressors/lbs
weights, ~130MB) dominate HBM traffic, so each core owns 1/8 of the SMPLX
vertices (plus the FLAME/MANO vertices its SMPLX rows stitch in) and computes
ALL B=128 batch elements for its shard.  The only cross-core dependency is the
joint regression J = J_regressor @ v_shaped -> one [76, 384] AllReduce of
partial joint sums.  FK (92 joints) is replicated on every core on the vector
engine with batch on partitions (B == 128 == n_partitions).

Per-vertex data layout: [vertex(partition<=128), (c, b)] with c-major free dim
(col = c*128 + b).  Batch-staged data (poses, FK, A matrices): [b(part), free].
"""

import sys

sys.path.insert(0, "/opt/trn_rl_repo")

from contextlib import ExitStack

import numpy as np
import ml_dtypes

BF16NP = ml_dtypes.bfloat16

import concourse.bass as bass
import concourse.bacc as bacc
import concourse.tile as tile
import concourse.mybir as mybir
from concourse.bass_utils import run_bass_kernel_spmd

F32 = mybir.dt.float32
BF16 = mybir.dt.bfloat16
AF = mybir.ActivationFunctionType
ALU = mybir.AluOpType

# ---------------------------------------------------------------- constants
B = 128
VS, VF, VM = 10475, 5023, 778
NL = 350
NCORES = 8

SMPLX_PARENTS = np.array([-1,0,0,0,1,2,3,4,5,6,7,8,9,9,9,12,13,14,16,17,18,19,
                          15,15,15,20,25,26,20,28,29,20,31,32,20,34,35,20,37,38,
                          21,40,41,21,43,44,21,46,47,21,49,50,21,52,53])
FLAME_PARENTS = np.array([-1,0,1,1,1])
MANO_PARENTS = np.array([-1,0,1,2,0,4,5,0,7,8,0,10,11,0,13,14])

N_PLAIN, N_HEAD, N_HL, N_HR = 768, 384, 128, 128
ROWS = N_PLAIN + N_HEAD + N_HL + N_HR        # 1408
NCH = ROWS // 128                            # 11
CH_PLAIN = set(range(0, 6))
CH_HEAD0 = 6                                 # chunks 6,7,8 head; 9 L; 10 R
CH_HL, CH_HR = 9, 10

NFE_CH = 5
NMJ_CH = 3
PD_S_K = 189
PD_F_K = 27
PD_M_K = 135

NJ_ALL = 92
OFF_S, OFF_F, OFF_L, OFF_R = 0, 55, 60, 76
NROT = 55
ROT_S0, ROT_F0, ROT_L0, ROT_R0 = 0, 22, 25, 40

BF16_INPUTS = {"w_s", "wre_f", "w_m",
               "sd_s", "pd_s_a", "pd_s_b", "jr_s", "sd_f", "jr_f", "pd_f",
               "sd_m", "pd_m_a", "pd_m_b", "sd_mj", "jreg_m",
               "betaT_s", "betaT_f", "betam"}


def _fk_forest():
    par = np.empty(NJ_ALL, np.int64)
    par[OFF_S:OFF_S + 55] = SMPLX_PARENTS
    par[OFF_F:OFF_F + 5] = np.where(FLAME_PARENTS < 0, -1, FLAME_PARENTS + OFF_F)
    par[OFF_L:OFF_L + 16] = np.where(MANO_PARENTS < 0, -1, MANO_PARENTS + OFF_L)
    par[OFF_R:OFF_R + 16] = np.where(MANO_PARENTS < 0, -1, MANO_PARENTS + OFF_R)
    return par


def _fk_levels(par):
    depth = np.zeros(NJ_ALL, np.int64)
    for j in range(NJ_ALL):
        if par[j] >= 0:
            depth[j] = depth[par[j]] + 1
    levels = []
    for d in range(1, int(depth.max()) + 1):
        js = np.nonzero(depth == d)[0]
        runs, i = [], 0
        while i < len(js):
            j0, p0 = int(js[i]), int(par[js[i]])
            if i + 1 < len(js):
                ds = int(js[i + 1]) - j0
                ps = int(par[js[i + 1]]) - p0
            else:
                ds, ps = 1, 0
            n = 1
            while (i + n < len(js) and int(js[i + n]) == j0 + n * ds
                   and int(par[js[i + n]]) == p0 + n * ps):
                n += 1
            if n == 1:
                ds, ps = 1, 0
            runs.append((j0, ds, n, p0, ps))
            i += n
        levels.append(runs)
    return levels


# ================================================================ host prep

def _split_sizes(total, parts):
    q, r = divmod(total, parts)
    return [q + (1 if i < r else 0) for i in range(parts)]


def _pad_ids(ids, n):
    out = np.full(n, -1, np.int64)
    out[:len(ids)] = ids
    return out


def _host_prep(inp):
    f32 = np.float32
    s2f = np.asarray(inp["smplx2flame_ind"])
    head_ix = np.asarray(inp["head_index"])
    s2l = np.asarray(inp["smplx2mano_left"])
    s2r = np.asarray(inp["smplx2mano_right"])

    head_sv = s2f[head_ix]
    special = np.zeros(VS, bool)
    special[head_sv] = True
    special[s2l] = True
    special[s2r] = True
    plain_sv = np.nonzero(~special)[0]

    pl_sp = np.cumsum([0] + _split_sizes(len(plain_sv), NCORES))
    hd_sp = np.cumsum([0] + _split_sizes(len(head_ix), NCORES))
    hl_sp = np.cumsum([0] + _split_sizes(VM, NCORES))
    fe_sp = np.cumsum([0] + _split_sizes(VF, NCORES))
    mj_sp = np.cumsum([0] + _split_sizes(VM * 3, NCORES))

    sd_s_np = np.asarray(inp["smplx_shapedirs"], f32)
    pd_s_np = np.asarray(inp["smplx_posedirs"], f32)
    jr_s_np = np.asarray(inp["smplx_J_regressor"], f32)
    w_s_np = np.asarray(inp["smplx_lbs_weights"], f32)
    tmpl_s = np.asarray(inp["smplx_v_template"], f32)
    sd_f_np = np.asarray(inp["flame_shapedirs"], f32)
    pd_f_np = np.asarray(inp["flame_posedirs"], f32)
    jr_f_np = np.asarray(inp["flame_J_regressor"], f32)
    w_f_np = np.asarray(inp["flame_lbs_weights"], f32)
    tmpl_f = np.asarray(inp["flame_v_template"], f32)
    re_np = np.asarray(inp["r_eyelid"], f32)
    le_np = np.asarray(inp["l_eyelid"], f32)
    sd_m_np = np.asarray(inp["mano_shapedirs"], f32)
    pd_m_np = np.asarray(inp["mano_posedirs"], f32)
    jr_m_np = np.asarray(inp["mano_J_regressor"], f32)
    w_m_np = np.asarray(inp["mano_lbs_weights"], f32)
    tmpl_m = np.asarray(inp["mano_v_template"], f32)

    aa = np.concatenate([
        np.asarray(inp["global_pose"], f32).reshape(B, 3),
        np.asarray(inp["body_pose"], f32).reshape(B, 63),
        np.asarray(inp["jaw_params"], f32).reshape(B, 3),
        np.asarray(inp["eye_pose"], f32).reshape(B, 6),
        np.asarray(inp["left_hand_pose"], f32).reshape(B, 45),
        np.asarray(inp["right_hand_pose"], f32).reshape(B, 45),
    ], axis=1)

    ep = np.asarray(inp["eyelid_params"], f32)
    aux = np.concatenate([
        np.asarray(inp["head_scale"], f32)[:, None],
        np.asarray(inp["left_hand_scale"], f32)[:, None],
        np.asarray(inp["right_hand_scale"], f32)[:, None],
        ep[:, 0:1], ep[:, 1:2],
        np.asarray(inp["head_pos_offset"], f32),
        np.asarray(inp["left_hand_pos_offset"], f32),
        np.asarray(inp["right_hand_pos_offset"], f32),
    ], axis=1)                                               # [128, 14]

    def beta_T(second):
        b = np.concatenate([np.asarray(inp["shape_params"], f32), second], 1)
        bt = np.zeros((384, B), f32)
        bt[:NL] = b.T
        bt[NL] = 1.0
        return bt.reshape(3, 128, B)

    betaT_s = beta_T(np.asarray(inp["body_exp"], f32))
    betaT_f = beta_T(np.asarray(inp["flame_exp"], f32))

    joff = np.asarray(inp["joints_offset"], f32)
    joffT = np.ascontiguousarray(joff.transpose(1, 2, 0)).reshape(55, 384)

    def mrel_T(par, nj):
        m = np.eye(nj, dtype=f32)
        for j in range(1, nj):
            if par[j] >= 0:
                m[j, par[j]] = -1.0
        return np.ascontiguousarray(m.T)

    betam = np.zeros((11, 1), f32)
    betam[:10, 0] = np.asarray(inp["mano_betas"], f32)[0]
    betam[10, 0] = 1.0

    sel16 = np.zeros((16, 16 * 128), f32)
    for s in range(16):
        sel16[s, s * 128:(s + 1) * 128] = 1.0
    rep = dict(aa=aa, aux=aux, betaT_s=betaT_s, betaT_f=betaT_f, joffT=joffT,
               mrelT_s=mrel_T(SMPLX_PARENTS, 55), mrelT_f=mrel_T(FLAME_PARENTS, 5),
               mrelT_m=mrel_T(MANO_PARENTS, 16), betam=betam,
               ident=np.eye(128, dtype=f32), sel16=sel16)

    sd_m_flat = sd_m_np.reshape(VM * 3, 10)
    tmpl_m_flat = tmpl_m.reshape(VM * 3)

    in_maps = []
    vid_all = np.full((NCORES, ROWS), -1, np.int64)

    for c in range(NCORES):
        p_ids = plain_sv[pl_sp[c]:pl_sp[c + 1]]
        h_pos = np.arange(hd_sp[c], hd_sp[c + 1])
        h_sv, h_fv = head_sv[h_pos], head_ix[h_pos]
        l_pos = np.arange(hl_sp[c], hl_sp[c + 1])
        r_pos = l_pos                                         # same split for R
        l_sv, r_sv = s2l[l_pos], s2r[r_pos]

        vid = np.full(ROWS, -1, np.int64)
        vid[:len(p_ids)] = p_ids
        vid[N_PLAIN:N_PLAIN + len(h_sv)] = h_sv
        vid[N_PLAIN + N_HEAD:N_PLAIN + N_HEAD + len(l_sv)] = l_sv
        vid[N_PLAIN + N_HEAD + N_HL:N_PLAIN + N_HEAD + N_HL + len(r_sv)] = r_sv
        vid_all[c] = vid
        vok = vid >= 0
        vc = np.where(vok, vid, 0)

        # smplx shapedirs slab [NCH, 128(p=l), (c, lk, v)]
        sdp = np.zeros((ROWS, 3, 384), f32)
        sdp[:, :, :NL] = np.where(vok[:, None, None], sd_s_np[vc], 0.0)
        sdp[:, :, NL] = np.where(vok[:, None], tmpl_s[vc], 0.0)
        slab = sdp.reshape(NCH, 128, 3, 3, 128).transpose(0, 4, 2, 3, 1)
        sd_s = np.ascontiguousarray(slab).reshape(NCH, 128, 1152)

        colv = vc[:, None] * 3 + np.arange(3)[None, :]
        pdv = pd_s_np[:PD_S_K][:, colv]
        pdv = np.where(vok[None, :, None], pdv, 0.0)
        pdv = pdv.reshape(PD_S_K, NCH, 128, 3).transpose(1, 0, 3, 2)
        pd_s_a = np.ascontiguousarray(pdv[:, :128]).reshape(NCH, 128, 384)
        pd_s_b = np.ascontiguousarray(pdv[:, 128:]).reshape(NCH, PD_S_K - 128, 384)

        jr_s = np.ascontiguousarray(
            np.where(vok[:, None], jr_s_np[:, vc].T, 0.0).reshape(NCH, 128, 55))
        w_s = np.ascontiguousarray(
            np.where(vok[:, None], w_s_np[vc], 0.0)
            .reshape(NCH, 128, 55).transpose(0, 2, 1))

        # flame: 5 even + 3 gathered chunks
        fe = _pad_ids(np.arange(fe_sp[c], fe_sp[c + 1]), NFE_CH * 128)
        fg = _pad_ids(h_fv, N_HEAD)
        f_rows = np.concatenate([fe, fg])
        fok = f_rows >= 0
        fc = np.where(fok, f_rows, 0)
        sdfp = np.zeros((len(f_rows), 3, 384), f32)
        sdfp[:, :, :NL] = np.where(fok[:, None, None], sd_f_np[fc], 0.0)
        sdfp[:, :, NL] = np.where(fok[:, None], tmpl_f[fc], 0.0)
        slab = sdfp.reshape(-1, 128, 3, 3, 128).transpose(0, 4, 2, 3, 1)
        sd_f = np.ascontiguousarray(slab).reshape(-1, 128, 1152)

        jr_f = np.ascontiguousarray(
            np.where(fok[:NFE_CH * 128, None], jr_f_np[:, fc[:NFE_CH * 128]].T, 0.0)
            .reshape(NFE_CH, 128, 5))

        fgc, fgok = fc[NFE_CH * 128:], fok[NFE_CH * 128:]
        colf = fgc[:, None] * 3 + np.arange(3)[None, :]
        pdfv = pd_f_np[9:36][:, colf]
        pdfv = np.where(fgok[None, :, None], pdfv, 0.0)
        pdfv = pdfv.reshape(PD_F_K, 3, 128, 3).transpose(1, 0, 3, 2)
        pd_f = np.ascontiguousarray(pdfv).reshape(3, PD_F_K, 384)

        wre = np.zeros((3, 11, 128), f32)
        for k in range(3):
            rows, ok = fgc[k * 128:(k + 1) * 128], fgok[k * 128:(k + 1) * 128]
            wre[k, :5] = np.where(ok[None, :], w_f_np[rows].T, 0.0)
            wre[k, 5:8] = np.where(ok[None, :], re_np[rows].T, 0.0)
            wre[k, 8:11] = np.where(ok[None, :], le_np[rows].T, 0.0)

        # mano hands + J shard
        m_rows = np.stack([_pad_ids(l_pos, 128), _pad_ids(r_pos, 128)])
        mok = m_rows >= 0
        mc = np.where(mok, m_rows, 0)
        sd_m = np.zeros((2, 11, 384), f32)
        pd_m_a = np.zeros((2, 128, 384), f32)
        pd_m_b = np.zeros((2, PD_M_K - 128, 384), f32)
        w_m = np.zeros((2, 16, 128), f32)
        for h in range(2):
            sdm = np.where(mok[h][:, None, None], sd_m_np[mc[h]], 0.0)
            sd_m[h, :10] = sdm.transpose(2, 1, 0).reshape(10, 384)
            sd_m[h, 10] = np.where(mok[h][:, None], tmpl_m[mc[h]], 0.0).T.reshape(384)
            colm = mc[h][:, None] * 3 + np.arange(3)[None, :]
            pdm = pd_m_np[:, colm]
            pdm = np.where(mok[h][None, :, None], pdm, 0.0).transpose(0, 2, 1)
            pd_m_a[h] = pdm[:128].reshape(128, 384)
            pd_m_b[h] = pdm[128:].reshape(PD_M_K - 128, 384)
            w_m[h] = np.where(mok[h][None, :], w_m_np[mc[h]].T, 0.0)

        mj = _pad_ids(np.arange(mj_sp[c], mj_sp[c + 1]), NMJ_CH * 128)
        mjok = mj >= 0
        mjc = np.where(mjok, mj, 0)
        sd_mj = np.concatenate(
            [np.where(mjok[:, None], sd_m_flat[mjc], 0.0),
             np.where(mjok, tmpl_m_flat[mjc], 0.0)[:, None]], 1)
        sd_mj = np.ascontiguousarray(
            sd_mj.reshape(NMJ_CH, 128, 11).transpose(0, 2, 1))
        jreg_m = np.zeros((NMJ_CH * 128, 48), f32)
        vv, cc3 = mjc // 3, mjc % 3
        jj = np.arange(16)
        jreg_m[np.arange(NMJ_CH * 128)[:, None], jj[None, :] * 3 + cc3[:, None]] = \
            np.where(mjok[:, None], jr_m_np[:, vv].T, 0.0)
        jreg_m = jreg_m.reshape(NMJ_CH, 128, 48)

        m = dict(rep)
        m.update(sd_s=sd_s, pd_s_a=pd_s_a, pd_s_b=pd_s_b, jr_s=jr_s, w_s=w_s,
                 sd_f=sd_f, jr_f=jr_f, pd_f=pd_f, wre_f=wre,
                 sd_m=sd_m, pd_m_a=pd_m_a, pd_m_b=pd_m_b, w_m=w_m,
                 sd_mj=sd_mj, jreg_m=jreg_m)
        out = {}
        for k, v in m.items():
            if k in BF16_INPUTS:
                out[k] = np.ascontiguousarray(v.astype(BF16NP))
            else:
                out[k] = np.ascontiguousarray(v, f32)
        in_maps.append(out)

    return in_maps, vid_all


# ================================================================ device IR

def _build_nc():
    nc = bacc.Bacc("TRN2", target_bir_lowering=False, debug=False,
                   num_devices=NCORES)
    di = {}

    def din(name, shape):
        dt = BF16 if name in BF16_INPUTS else F32
        di[name] = nc.dram_tensor(name, list(shape), dt, kind="ExternalInput").ap()

    din("aa", (B, 165)); din("aux", (B, 14))
    din("betaT_s", (3, 128, 128)); din("betaT_f", (3, 128, 128))
    din("joffT", (55, 384))
    din("mrelT_s", (55, 55)); din("mrelT_f", (5, 5)); din("mrelT_m", (16, 16))
    din("betam", (11, 1)); din("ident", (128, 128)); din("sel16", (16, 2048))
    din("sd_s", (NCH, 128, 1152)); din("pd_s_a", (NCH, 128, 384))
    din("pd_s_b", (NCH, PD_S_K - 128, 384))
    din("jr_s", (NCH, 128, 55)); din("w_s", (NCH, 55, 128))
    din("sd_f", (8, 128, 1152)); din("jr_f", (NFE_CH, 128, 5))
    din("pd_f", (3, PD_F_K, 384)); din("wre_f", (3, 11, 128))
    din("sd_m", (2, 11, 384)); din("pd_m_a", (2, 128, 384))
    din("pd_m_b", (2, PD_M_K - 128, 384)); din("w_m", (2, 16, 128))
    din("sd_mj", (NMJ_CH, 11, 128)); din("jreg_m", (NMJ_CH, 128, 48))

    out_d = nc.dram_tensor("out", [ROWS, 384], F32, kind="ExternalOutput").ap()
    dbg_d = None
    if DEBUG:
        dbg_d = nc.dram_tensor("dbg", [128, 4096], F32, kind="ExternalOutput").ap()

    with tile.TileContext(nc) as tc:
        _emit(nc, tc, di, out_d, dbg_d)
    nc.compile()
    return nc


def _emit(nc, tc, di, out_d, dbg_d=None):
    levels = _fk_levels(_fk_forest())
    es = ExitStack()
    persist = es.enter_context(tc.tile_pool(name="persist", bufs=1))
    slabs = es.enter_context(tc.tile_pool(name="slabs", bufs=3))
    acc_cm = tc.tile_pool(name="acc", bufs=2, space="PSUM")
    acc = acc_cm.__enter__()
    jpool_cm = tc.tile_pool(name="jpool", bufs=1, space="PSUM")
    jpool = jpool_cm.__enter__()
    dram = es.enter_context(tc.tile_pool(name="dram", bufs=1, space="DRAM"))

    V, S, G, T, DMA = nc.vector, nc.scalar, nc.gpsimd, nc.tensor, nc.sync

    def ptile(shape, name):
        return persist.tile(list(shape), F32, tag=name, name=name)

    # ---------------- constants / staged inputs --------------------------
    ident = ptile((128, 128), "ident")
    DMA.dma_start(ident[:], di["ident"][:])
    sel16 = ptile((16, 2048), "sel16")
    DMA.dma_start(sel16[:], di["sel16"][:])

    aa = ptile((B, 165), "aa"); DMA.dma_start(aa[:], di["aa"][:])
    aux = ptile((B, 14), "aux"); DMA.dma_start(aux[:], di["aux"][:])

    betaT_s = persist.tile([128, 384], BF16, tag="betaT_s", name="betaT_s")
    betaT_f = persist.tile([128, 384], BF16, tag="betaT_f", name="betaT_f")
    for lk in range(3):
        DMA.dma_start(betaT_s[:, lk * 128:(lk + 1) * 128], di["betaT_s"][lk])
        DMA.dma_start(betaT_f[:, lk * 128:(lk + 1) * 128], di["betaT_f"][lk])
    joffT = ptile((55, 384), "joffT"); DMA.dma_start(joffT[:], di["joffT"][:])
    mrelT_s = ptile((55, 55), "mrelT_s"); DMA.dma_start(mrelT_s[:], di["mrelT_s"][:])
    mrelT_f = ptile((5, 5), "mrelT_f"); DMA.dma_start(mrelT_f[:], di["mrelT_f"][:])
    mrelT_m = ptile((16, 16), "mrelT_m"); DMA.dma_start(mrelT_m[:], di["mrelT_m"][:])
    betam = persist.tile([11, 1], BF16, tag="betam", name="betam"); DMA.dma_start(betam[:], di["betam"][:])

    # ---------------- rodrigues ------------------------------------------
    rot = ptile((B, NROT * 9), "rot")
    _rodrigues(nc, aa, rot, ptile)
    rot4 = rot[:].rearrange("p (j x) -> p j x", x=9)

    def pf_make(name, j0, n):
        t = ptile((B, n * 9), name)
        t9 = t[:].rearrange("p (j x) -> p j x", x=9)
        V.tensor_copy(t9, rot4[:, j0:j0 + n, :])
        V.tensor_scalar_add(t9[:, :, 0:9:4], t9[:, :, 0:9:4], -1.0)
        return t

    pf_s = pf_make("pf_s", 1, 21)
    pf_f = pf_make("pf_f", 22, 3)
    pf_m = [pf_make("pf_l", 25, 15), pf_make("pf_r", 40, 15)]

    def transpose_to(dst_ap, src_ap):
        pp = acc.tile([128, 384], F32, tag="tpose", padded_shape=[128, 512])
        k, n = src_ap.shape[0], src_ap.shape[1]
        T.matmul(pp[:n, :k], src_ap, ident[:k, :k], is_transpose=True,
                 start=True, stop=True)
        S.copy(dst_ap, pp[:n, :k])

    pfT_s_a = persist.tile([128, 128], BF16, tag="pfT_s_a", name="pfT_s_a")
    pfT_s_b = persist.tile([PD_S_K - 128, 128], BF16, tag="pfT_s_b", name="pfT_s_b")
    transpose_to(pfT_s_a[:], pf_s[:, 0:128])
    transpose_to(pfT_s_b[:], pf_s[:, 128:PD_S_K])
    pfT_f = persist.tile([PD_F_K, 128], BF16, tag="pfT_f", name="pfT_f")
    transpose_to(pfT_f[:], pf_f[:, :])
    pfT_m_a = [persist.tile([128, 128], BF16, tag="pfT_l_a", name="pfT_l_a"), persist.tile([128, 128], BF16, tag="pfT_r_a", name="pfT_r_a")]
    pfT_m_b = [persist.tile([PD_M_K - 128, 128], BF16, tag="pfT_l_b", name="pfT_l_b"),
               persist.tile([PD_M_K - 128, 128], BF16, tag="pfT_r_b", name="pfT_r_b")]
    for h in range(2):
        transpose_to(pfT_m_a[h][:], pf_m[h][:, 0:128])
        transpose_to(pfT_m_b[h][:], pf_m[h][:, 128:PD_M_K])

    # ---------------- stage A: blend shapes + J partials ------------------
    jpt = jpool.tile([128, 1536], F32, tag="jpsum", name="jpt")
    jps = jpt[:, 0:384]
    jps_f = jpt[:, 512:896]
    jps_m = jpt[:, 1024:1025]

    vp_sbuf = [ptile((128, 384), f"vp{i}") for i in range(NCH)]
    vpf_sbuf = [ptile((128, 384), f"vpf{h}") for h in range(3)]
    vpm_sbuf = [ptile((128, 384), f"vpm{h}") for h in range(2)]

    def sd_mms(pp, slab_t, betaT, last=True):
        for c3 in range(3):
            for lk in range(3):
                T.matmul(pp[:, c3 * 128:(c3 + 1) * 128],
                         slab_t[:, (c3 * 3 + lk) * 128:(c3 * 3 + lk + 1) * 128],
                         betaT[:, lk * 128:(lk + 1) * 128],
                         start=(lk == 0), stop=(lk == 2 and last))

    vsb = [persist.tile([128, 384], BF16, tag=f"vsb{i}", name=f"vsb{i}")
           for i in range(NCH)]
    vsf32 = {i: ptile((128, 384), f"vsf32{i}") for i in CH_PLAIN}

    # ---- A1: shape blend + J partials (everything the AllReduce needs) ----
    for i in range(NCH):
        sdt = slabs.tile((128, 1152), BF16, tag="sd_s")
        DMA.dma_start(sdt[:], di["sd_s"][i])
        pp = acc.tile([128, 384], F32, tag="vppsum", padded_shape=[128, 512])
        sd_mms(pp, sdt, betaT_s)
        S.copy(vsb[i][:], pp[:])
        if i in CH_PLAIN:
            S.copy(vsf32[i][:], pp[:])
        jrt = slabs.tile((128, 55), BF16, tag="jr_s")
        DMA.dma_start(jrt[:], di["jr_s"][i])
        T.matmul(jps[0:55, :], jrt[:], vsb[i][:], start=(i == 0),
                 stop=(i == NCH - 1))

    for k in range(NFE_CH):
        sdt = slabs.tile((128, 1152), BF16, tag="sd_f")
        DMA.dma_start(sdt[:], di["sd_f"][k])
        pp = acc.tile([128, 384], F32, tag="vppsum", padded_shape=[128, 512])
        sd_mms(pp, sdt, betaT_f)
        vsf = slabs.tile((128, 384), BF16, tag="vsf")
        S.copy(vsf[:], pp[:])
        jrt = slabs.tile((128, 5), BF16, tag="jr_f")
        DMA.dma_start(jrt[:], di["jr_f"][k])
        T.matmul(jps_f[0:5, 0:384], jrt[:], vsf[:],
                 start=(k == 0), stop=(k == NFE_CH - 1))

    for k in range(NMJ_CH):
        sdt = slabs.tile((11, 128), BF16, tag="sd_mj")
        DMA.dma_start(sdt[:], di["sd_mj"][k])
        pp = acc.tile([128, 384], F32, tag="vppsum", padded_shape=[128, 512])
        T.matmul(pp[:, 0:1], sdt[:], betam[:], start=True, stop=True)
        vsm = slabs.tile((128, 1), BF16, tag="vsmj")
        S.copy(vsm[:], pp[:, 0:1])
        jrt = slabs.tile((128, 48), BF16, tag="jreg_m")
        DMA.dma_start(jrt[:], di["jreg_m"][k])
        T.matmul(jps_m[0:48, 0:1], jrt[:], vsm[:],
                 start=(k == 0), stop=(k == NMJ_CH - 1))

    # ---- J AllReduce (launch早 so A2 work overlaps it) -------------------
    jsb = ptile((128, 384), "jsb")
    G.memset(jsb[:], 0.0)
    S.copy(jsb[0:55, :], jps[0:55, :])
    jsb_f = ptile((5, 384), "jsb_f")
    S.copy(jsb_f[:], jps_f[0:5, 0:384])
    jsb_m = ptile((48, 1), "jsb_m")
    S.copy(jsb_m[:], jps_m[0:48, 0:1])
    jpool_cm.__exit__(None, None, None)
    ar_in = dram.tile([76, 384], F32, tag="ar_in")
    ar_out = dram.tile([76, 384], F32, tag="ar_out")
    DMA.dma_start(ar_in[0:55, :], jsb[0:55, :])
    DMA.dma_start(ar_in[55:60, :], jsb_f[:])
    DMA.dma_start(ar_in[60:76, :], jsb[60:76, :])
    DMA.dma_start(ar_in[60:76, 0:3], jsb_m[:])
    G.collective_compute("AllReduce", ALU.add,
                         replica_groups=[list(range(NCORES))],
                         ins=[ar_in[:].opt()], outs=[ar_out[:].opt()])
    arr = ptile((55, 384), "arr")
    arr_f = ptile((5, 384), "arr_f")
    arr_m = ptile((16, 3), "arr_m")
    DMA.dma_start(arr[:], ar_out[0:55, :])
    DMA.dma_start(arr_f[:], ar_out[55:60, :])
    DMA.dma_start(arr_m[:], ar_out[60:76, 0:3])

    # ---- A2: posedirs + flame/mano v_posed (overlaps the AllReduce) ------
    for i in range(NCH):
        pda = slabs.tile((128, 384), BF16, tag="pd_s_a")
        pdb = slabs.tile((PD_S_K - 128, 384), BF16, tag="pd_s_b")
        DMA.dma_start(pda[:], di["pd_s_a"][i])
        DMA.dma_start(pdb[:], di["pd_s_b"][i])
        pq = acc.tile([128, 384], F32, tag="vppsum", padded_shape=[128, 512])
        for c3 in range(3):
            T.matmul(pq[:, c3 * 128:(c3 + 1) * 128],
                     pda[:, c3 * 128:(c3 + 1) * 128], pfT_s_a[:],
                     start=True, stop=False)
            T.matmul(pq[:, c3 * 128:(c3 + 1) * 128],
                     pdb[:, c3 * 128:(c3 + 1) * 128], pfT_s_b[:],
                     start=False, stop=True)
        if i in CH_PLAIN:
            V.tensor_add(vp_sbuf[i][:], vsf32[i][:], pq[:])
        else:
            S.copy(vp_sbuf[i][:], pq[:])

    for h in range(3):
        sdt = slabs.tile((128, 1152), BF16, tag="sd_f")
        DMA.dma_start(sdt[:], di["sd_f"][NFE_CH + h])
        pp = acc.tile([128, 384], F32, tag="vppsum", padded_shape=[128, 512])
        pdf = slabs.tile((PD_F_K, 384), BF16, tag="pd_f")
        DMA.dma_start(pdf[:], di["pd_f"][h])
        for c3 in range(3):
            for lk in range(3):
                T.matmul(pp[:, c3 * 128:(c3 + 1) * 128],
                         sdt[:, (c3 * 3 + lk) * 128:(c3 * 3 + lk + 1) * 128],
                         betaT_f[:, lk * 128:(lk + 1) * 128],
                         start=(lk == 0), stop=False)
            T.matmul(pp[:, c3 * 128:(c3 + 1) * 128],
                     pdf[:, c3 * 128:(c3 + 1) * 128], pfT_f[:],
                     start=False, stop=True)
        S.copy(vpf_sbuf[h][:], pp[:])

    for h in range(2):
        sdt = slabs.tile((11, 384), BF16, tag="sd_m")
        DMA.dma_start(sdt[:], di["sd_m"][h])
        pps = acc.tile([128, 384], F32, tag="vppsum", padded_shape=[128, 512])
        for c3 in range(3):
            T.matmul(pps[:, c3:c3 + 1], sdt[:, c3 * 128:(c3 + 1) * 128],
                     betam[:], start=True, stop=True)
        vshm = ptile((128, 3), f"vshm{h}")
        S.copy(vshm[:], pps[:, 0:3])
        pda = slabs.tile((128, 384), BF16, tag="pd_m_a")
        pdb = slabs.tile((PD_M_K - 128, 384), BF16, tag="pd_m_b")
        DMA.dma_start(pda[:], di["pd_m_a"][h])
        DMA.dma_start(pdb[:], di["pd_m_b"][h])
        pq = acc.tile([128, 384], F32, tag="vppsum", padded_shape=[128, 512])
        for c3 in range(3):
            T.matmul(pq[:, c3 * 128:(c3 + 1) * 128],
                     pda[:, c3 * 128:(c3 + 1) * 128], pfT_m_a[h][:],
                     start=True, stop=False)
            T.matmul(pq[:, c3 * 128:(c3 + 1) * 128],
                     pdb[:, c3 * 128:(c3 + 1) * 128], pfT_m_b[h][:],
                     start=False, stop=True)
        vpm = vpm_sbuf[h]
        V.tensor_add(vpm[:].rearrange("p (c b) -> p c b", b=128),
                     pq[:].rearrange("p (c b) -> p c b", b=128),
                     vshm[:].unsqueeze(2).broadcast_to([128, 3, 128]))

    # ---------------- joints, FK, A matrices ------------------------------
    tbj = ptile((55, 384), "tbj")
    V.tensor_add(tbj[:], arr[0:55, :], joffT[:])

    rel_s = ptile((55, 384), "rel_s")
    pp = acc.tile([128, 384], F32, tag="tpose", padded_shape=[128, 512])
    T.matmul(pp[0:55, :], mrelT_s[:], tbj[:], start=True, stop=True)
    S.copy(rel_s[:], pp[0:55, :])
    rel_f = ptile((5, 384), "rel_f")
    pp = acc.tile([128, 384], F32, tag="tpose", padded_shape=[128, 512])
    T.matmul(pp[0:5, :], mrelT_f[:], arr_f[:], start=True, stop=True)
    S.copy(rel_f[:], pp[0:5, :])
    rel_m = ptile((16, 3), "rel_m")
    pp = acc.tile([128, 384], F32, tag="tpose", padded_shape=[128, 512])
    T.matmul(pp[0:16, 0:3], mrelT_m[:], arr_m[:], start=True, stop=True)
    S.copy(rel_m[:], pp[0:16, 0:3])

    jb = ptile((B, 165), "jb")
    relb = ptile((B, 165), "relb")
    jfb = ptile((B, 15), "jfb")
    relfb = ptile((B, 15), "relfb")
    for c3 in range(3):
        transpose_to(jb[:, c3 * 55:(c3 + 1) * 55], tbj[:, c3 * 128:(c3 + 1) * 128])
        transpose_to(relb[:, c3 * 55:(c3 + 1) * 55], rel_s[:, c3 * 128:(c3 + 1) * 128])
        transpose_to(jfb[:, c3 * 5:(c3 + 1) * 5], arr_f[:, c3 * 128:(c3 + 1) * 128])
        transpose_to(relfb[:, c3 * 5:(c3 + 1) * 5], rel_f[:, c3 * 128:(c3 + 1) * 128])

    jmT = ptile((3, 16), "jmT")
    relmT = ptile((3, 16), "relmT")
    transpose_to(jmT[:], arr_m[:])
    transpose_to(relmT[:], rel_m[:])
    jmb = ptile((B, 48), "jmb")
    relmb = ptile((B, 48), "relmb")
    pb = acc.tile([128, 384], F32, tag="tpose", padded_shape=[128, 512])
    for c3 in range(3):
        T.matmul(pb[:, c3 * 16:(c3 + 1) * 16],
                 sel16[0:3, c3 * 128:(c3 + 1) * 128], jmT[:],
                 start=True, stop=True)
        T.matmul(pb[:, 48 + c3 * 16:48 + (c3 + 1) * 16],
                 sel16[0:3, c3 * 128:(c3 + 1) * 128], relmT[:],
                 start=True, stop=True)
    S.copy(jmb[:], pb[:, 0:48])
    S.copy(relmb[:], pb[:, 48:96])

    # ---- FK ----
    Tb = ptile((B, NJ_ALL * 12), "Tb")
    Ab = ptile((B, NJ_ALL * 12), "Ab")
    T4 = Tb[:].rearrange("p (j m n) -> p j m n", m=3, n=4)
    A4 = Ab[:].rearrange("p (j m n) -> p j m n", m=3, n=4)
    G.memset(Tb[:], 0.0)

    def rot_to_T(tj0, rj0, n):
        V.tensor_copy(T4[:, tj0:tj0 + n, :, 0:3],
                      rot4[:, rj0:rj0 + n, :].rearrange("p j (m n) -> p j m n", n=3))

    rot_to_T(0, ROT_S0, 22)
    rot_to_T(OFF_F + 2, ROT_F0, 3)
    rot_to_T(OFF_L + 1, ROT_L0, 15)
    rot_to_T(OFF_R + 1, ROT_R0, 15)
    for j0, n in ((22, 33), (OFF_F, 2), (OFF_L, 1), (OFF_R, 1)):
        G.memset(Tb[:].rearrange("p (j x) -> p j x", x=12)[:, j0:j0 + n, 0:11:5], 1.0)
    V.tensor_copy(T4[:, 0:55, :, 3], relb[:].rearrange("p (c j) -> p j c", c=3))
    V.tensor_copy(T4[:, OFF_F:OFF_F + 5, :, 3],
                  relfb[:].rearrange("p (c j) -> p j c", c=3))
    for off in (OFF_L, OFF_R):
        V.tensor_copy(T4[:, off:off + 16, :, 3],
                      relmb[:].rearrange("p (c j) -> p j c", c=3))

    for r in (0, OFF_F, OFF_L, OFF_R):
        V.tensor_copy(A4[:, r:r + 1], T4[:, r:r + 1])
    fk_tmp = ptile((B, 12 * 16), "fk_tmp")
    fk_tmp2 = ptile((B, 12 * 16), "fk_tmp2")
    for runs in levels:
        for (d0, ds, n, p0, ps) in runs:
            sl_d = slice(d0, d0 + (n - 1) * ds + 1, ds) if ds != 1 else slice(d0, d0 + n)
            dst, dT = A4[:, sl_d], T4[:, sl_d]
            if ps == 0:
                par = A4[:, p0:p0 + 1].broadcast_to([B, n, 3, 4])
            else:
                sl_p = slice(p0, p0 + (n - 1) * ps + 1, ps) if ps != 1 else slice(p0, p0 + n)
                par = A4[:, sl_p]
            tmp = fk_tmp[:].rearrange("p (j m n) -> p j m n", m=3, n=4)[:, :n]
            sc2 = fk_tmp2[:].rearrange("p (j m n) -> p j m n", m=3, n=4)[:, :n]
            for k in range(3):
                a_k = par[:, :, :, k:k + 1].broadcast_to([B, n, 3, 4])
                t_k = dT[:, :, k:k + 1, :].broadcast_to([B, n, 3, 4])
                if k == 0:
                    V.tensor_mul(tmp, a_k, t_k)
                else:
                    V.tensor_mul(sc2, a_k, t_k)
                    V.tensor_add(tmp, tmp, sc2)
            V.tensor_add(tmp[:, :, :, 3], tmp[:, :, :, 3], par[:, :, :, 3])
            V.tensor_copy(dst, tmp)

    # ---- per-batch staging (world translations BEFORE rel-correction) ----
    hm = ptile((B, 16), "hm")
    jb3 = jb[:].rearrange("p (c j) -> p c j", c=3)
    jm3 = jmb[:].rearrange("p (c j) -> p c j", c=3)
    bias9 = ptile((B, 9), "bias9")
    V.tensor_add(hm[:, 0:3], jb3[:, :, 23], jb3[:, :, 24])
    V.tensor_add(hm[:, 3:6], A4[:, OFF_F + 3, :, 3], A4[:, OFF_F + 4, :, 3])
    V.tensor_sub(hm[:, 6:9], hm[:, 0:3], hm[:, 3:6])
    V.tensor_scalar_mul(hm[:, 6:9], hm[:, 6:9], 0.5)
    V.tensor_add(bias9[:, 0:3], hm[:, 6:9], aux[:, 5:8])
    V.tensor_sub(hm[:, 9:12], aux[:, 8:11], jm3[:, :, 0])
    V.tensor_sub(bias9[:, 3:4], jb3[:, 0:1, 20], hm[:, 9:10])
    V.tensor_add(bias9[:, 4:6], hm[:, 10:12], jb3[:, 1:3, 20])
    V.tensor_sub(hm[:, 12:15], aux[:, 11:14], jm3[:, :, 0])
    V.tensor_add(bias9[:, 6:9], hm[:, 12:15], jb3[:, :, 21])
    epp = ptile((B, 2), "epp")
    V.tensor_mul(epp[:], aux[:, 3:5], aux[:, 0:1].broadcast_to([B, 2]))

    # ---- A_rel: translation -= R_world @ J ------------------------------
    corr_tmp = ptile((B, NJ_ALL * 3), "corr_tmp")
    corr_tmp2 = ptile((B, NJ_ALL * 3), "corr_tmp2")

    def corr(j0, nj, jsrc):
        ct = corr_tmp[:].rearrange("p (j m) -> p j m", m=3)[:, 0:nj]
        ct2 = corr_tmp2[:].rearrange("p (j m) -> p j m", m=3)[:, 0:nj]
        js = jsrc.rearrange("p (c j) -> p c j", c=3)
        for k in range(3):
            a_k = A4[:, j0:j0 + nj, :, k]
            j_k = js[:, k, :].unsqueeze(2).broadcast_to([B, nj, 3])
            if k == 0:
                V.tensor_mul(ct, a_k, j_k)
            else:
                V.tensor_mul(ct2, a_k, j_k)
                V.tensor_add(ct, ct, ct2)
        V.tensor_sub(A4[:, j0:j0 + nj, :, 3], A4[:, j0:j0 + nj, :, 3], ct)

    corr(OFF_S, 55, jb[:])
    corr(OFF_F, 5, jfb[:])
    corr(OFF_L, 16, jmb[:])
    corr(OFF_R, 16, jmb[:])

    # ---- scale folding --------------------------------------------------
    V.tensor_scalar_mul(Ab[:, OFF_F * 12:(OFF_F + 5) * 12],
                        Ab[:, OFF_F * 12:(OFF_F + 5) * 12], aux[:, 0:1])
    negls = ptile((B, 1), "negls")
    V.tensor_scalar_mul(negls[:], aux[:, 1:2], -1.0)
    AL = A4[:, OFF_L:OFF_L + 16]
    V.tensor_scalar_mul(AL[:, :, 0, :], AL[:, :, 0, :], negls[:, 0:1])
    V.tensor_scalar_mul(AL[:, :, 1:3, :], AL[:, :, 1:3, :], aux[:, 1:2])
    ARr = A4[:, OFF_R:OFF_R + 16]
    V.tensor_scalar_mul(ARr[:, :, :, :], ARr[:, :, :, :], aux[:, 2:3])

    # ---- rhs assembly: [K, (n, m, b)] -----------------------------------
    def rhs_fill(rhs_t, j0, nj, col0, n4):
        pp = acc.tile([128, 384], F32, tag="tpose", padded_shape=[128, 512])
        for m3 in range(3):
            T.matmul(pp[0:nj, m3 * 128:(m3 + 1) * 128],
                     A4[:, j0:j0 + nj, m3, n4], ident[:],
                     is_transpose=True, start=True, stop=True)
        S.copy(rhs_t[0:nj, col0:col0 + 384], pp[0:nj, 0:384])

    rhs_s = persist.tile([55, 1536], BF16, tag="rhs_s", name="rhs_s")
    for n4 in range(4):
        rhs_fill(rhs_s, 0, 55, n4 * 384, n4)

    rhs_f = persist.tile([11, 2304], BF16, tag="rhs_f", name="rhs_f")
    G.memset(rhs_f[:], 0.0)
    for n4 in range(4):
        rhs_fill(rhs_f, OFF_F, 5, n4 * 384, n4)
    epT = persist.tile([2, 128], BF16, tag="epT", name="epT")
    transpose_to(epT[:], epp[:, :])
    for m3 in range(3):
        DMA.dma_start(rhs_f[5 + m3:6 + m3, (12 + m3) * 128:(13 + m3) * 128],
                      epT[1:2, :])
        DMA.dma_start(rhs_f[8 + m3:9 + m3, (15 + m3) * 128:(16 + m3) * 128],
                      epT[0:1, :])

    rhs_m = [persist.tile([16, 1536], BF16, tag="rhs_l", name="rhs_l"), persist.tile([16, 1536], BF16, tag="rhs_r", name="rhs_r")]
    for h, off in ((0, OFF_L), (1, OFF_R)):
        for n4 in range(4):
            rhs_fill(rhs_m[h], off, 16, n4 * 384, n4)

    bias9T = ptile((9, 128), "bias9T")
    transpose_to(bias9T[:], bias9[:, :])
    bcast = [ptile((128, 384), f"bcast{t}") for t in range(3)]
    for t in range(3):
        pp = acc.tile([128, 384], F32, tag="tpose", padded_shape=[128, 512])
        for c3 in range(3):
            T.matmul(pp[:, c3 * 128:(c3 + 1) * 128],
                     sel16[0:9, (t * 3 + c3) * 128:(t * 3 + c3 + 1) * 128],
                     bias9T[:], start=True, stop=True)
        S.copy(bcast[t][:], pp[:])

    if dbg_d is not None:
        DMA.dma_start(dbg_d[0:11, 0:2304], rhs_f[:])
        DMA.dma_start(dbg_d[16:19, 0:128], pfT_f[0:3, 0:128])
        for h in range(3):
            DMA.dma_start(dbg_d[32 + h:33 + h, 0:384],
                          vpf_sbuf[h][0:1, 0:384])
        DMA.dma_start(dbg_d[40:95, 0:1536], rhs_s[:])
    acc_cm.__exit__(None, None, None)
    big_cm = tc.tile_pool(name="big", bufs=2, space="PSUM")
    big = big_cm.__enter__()

    # ---------------- skinning per chunk ---------------------------------
    def t_apply(dst_ap, tpsum_ap, x_sbuf, scratch):
        """dst = sum_{n<3} T'[n]*x_n + T'[3]; layouts (n, m, b)."""
        d3 = dst_ap.rearrange("p (m b) -> p m b", b=128)
        x3 = x_sbuf[:].rearrange("p (c b) -> p c b", b=128)
        tp = tpsum_ap.rearrange("p (n m b) -> p n m b", m=3, b=128)
        sc = scratch.rearrange("p (m b) -> p m b", b=128)
        V.tensor_mul(d3, tp[:, 0], x3[:, 0:1].broadcast_to([128, 3, 128]))
        for n4 in (1, 2):
            V.tensor_mul(sc, tp[:, n4], x3[:, n4:n4 + 1].broadcast_to([128, 3, 128]))
            V.tensor_add(d3, d3, sc)
        V.tensor_add(d3, d3, tp[:, 3])

    scr_t = [ptile((128, 384), f"scr{i}") for i in range(4)]

    for i in range(NCH):
        if CH_HEAD0 <= i < CH_HEAD0 + 3:
            h = i - CH_HEAD0
            hv = slabs.tile((128, 384), F32, tag="hv", bufs=2, name="hv")
            wt = slabs.tile((11, 128), BF16, tag="wre_f")
            DMA.dma_start(wt[:], di["wre_f"][h])
            tp1 = big.tile([128, 1536], F32, tag="bigp")
            for g, w in ((0, 512), (1, 512), (2, 128)):
                T.matmul(tp1[:, g * 512:g * 512 + w], wt[:],
                         rhs_f[:, g * 512:g * 512 + w], start=True, stop=True)
            d3 = hv[:].rearrange("p (m b) -> p m b", b=128)
            x3 = vpf_sbuf[h][:].rearrange("p (c b) -> p c b", b=128)
            t1 = tp1[:].rearrange("p (n m b) -> p n m b", m=3, b=128)
            sc = scr_t[i % 4][:].rearrange("p (m b) -> p m b", b=128)
            V.tensor_mul(d3, t1[:, 0], x3[:, 0:1].broadcast_to([128, 3, 128]))
            for n4 in (1, 2):
                V.tensor_mul(sc, t1[:, n4], x3[:, n4:n4 + 1].broadcast_to([128, 3, 128]))
                V.tensor_add(d3, d3, sc)
            tp2 = big.tile([128, 1536], F32, tag="bigp")
            for g, w in ((0, 512), (1, 512), (2, 128)):
                T.matmul(tp2[:, g * 512:g * 512 + w], wt[:],
                         rhs_f[:, 1152 + g * 512:1152 + g * 512 + w],
                         start=True, stop=True)
            t2 = tp2[:].rearrange("p (n m b) -> p n m b", m=3, b=128)
            for e in range(3):
                V.tensor_add(d3, d3, t2[:, e])
            V.tensor_add(hv[:], hv[:], bcast[0][:])
            V.tensor_add(vp_sbuf[i][:], vp_sbuf[i][:], hv[:])
        elif i in (CH_HL, CH_HR):
            h = i - CH_HL
            hv = slabs.tile((128, 384), F32, tag="hv", bufs=2, name="hv")
            wt = slabs.tile((16, 128), BF16, tag="w_m")
            DMA.dma_start(wt[:], di["w_m"][h])
            tpm = big.tile([128, 1536], F32, tag="bigp")
            for g in range(3):
                T.matmul(tpm[:, g * 512:(g + 1) * 512], wt[:],
                         rhs_m[h][:, g * 512:(g + 1) * 512], start=True, stop=True)
            t_apply(hv[:], tpm[:], vpm_sbuf[h], scr_t[i % 4][:])
            V.tensor_add(hv[:], hv[:], bcast[1 + h][:])
            V.tensor_add(vp_sbuf[i][:], vp_sbuf[i][:], hv[:])

        wt = slabs.tile((55, 128), BF16, tag="w_s")
        DMA.dma_start(wt[:], di["w_s"][i])
        tps = big.tile([128, 1536], F32, tag="bigp")
        for g in range(3):
            T.matmul(tps[:, g * 512:(g + 1) * 512], wt[:],
                     rhs_s[:, g * 512:(g + 1) * 512], start=True, stop=True)
        ot = slabs.tile((128, 384), F32, tag="outt", bufs=3, name="ot")
        t_apply(ot[:], tps[:], vp_sbuf[i], scr_t[i % 4][:])
        DMA.dma_start(out_d[i * 128:(i + 1) * 128, :], ot[:])

    big_cm.__exit__(None, None, None)
    es.close()


def _rodrigues(nc, aa, rot, ptile):
    V, S = nc.vector, nc.scalar
    J = NROT
    aa3 = aa[:].rearrange("p (j k) -> p j k", k=3)
    sq = ptile((B, J), "rg_sq")
    tmp = ptile((B, J), "rg_tmp")
    V.tensor_mul(sq[:], aa3[:, :, 0], aa3[:, :, 0])
    V.tensor_mul(tmp[:], aa3[:, :, 1], aa3[:, :, 1])
    V.tensor_add(sq[:], sq[:], tmp[:])
    V.tensor_mul(tmp[:], aa3[:, :, 2], aa3[:, :, 2])
    V.tensor_add(sq[:], sq[:], tmp[:])
    eps_t = ptile((B, 1), "rg_eps")
    nc.gpsimd.memset(eps_t[:], 1e-8)
    hpi_t = ptile((B, 1), "rg_hpi")
    nc.gpsimd.memset(hpi_t[:], float(np.pi / 2))
    zero_t = ptile((B, 1), "rg_zero")
    nc.gpsimd.memset(zero_t[:], 0.0)
    ang = ptile((B, J), "rg_ang")
    S.activation(ang[:], sq[:], AF.Sqrt, bias=eps_t[:])
    inv = ptile((B, J), "rg_inv")
    V.reciprocal(inv[:], ang[:])
    sn = ptile((B, J), "rg_sin")
    co = ptile((B, J), "rg_cos")
    S.activation(sn[:], ang[:], AF.Sin, bias=zero_t[:])
    S.activation(co[:], ang[:], AF.Sin, bias=hpi_t[:])
    nv = ptile((B, 3 * J), "rg_n")
    n3 = nv[:].rearrange("p (j k) -> p j k", k=3)
    V.tensor_mul(n3, aa3, inv[:].unsqueeze(2).broadcast_to([B, J, 3]))
    u = ptile((B, J), "rg_u")
    V.tensor_scalar(u[:], co[:], -1.0, 1.0, ALU.mult, ALU.add)
    un = ptile((B, 3 * J), "rg_un")
    un3 = un[:].rearrange("p (j k) -> p j k", k=3)
    V.tensor_mul(un3, n3, u[:].unsqueeze(2).broadcast_to([B, J, 3]))
    q = ptile((B, 3 * J), "rg_q")
    q3 = q[:].rearrange("p (j k) -> p j k", k=3)
    V.tensor_mul(q3, un3, n3)
    d = ptile((B, J), "rg_d")
    V.tensor_add(d[:], q3[:, :, 0], q3[:, :, 1])
    V.tensor_add(d[:], d[:], q3[:, :, 2])
    dd = ptile((B, J), "rg_dd")
    V.tensor_scalar(dd[:], d[:], -1.0, 1.0, ALU.mult, ALU.add)
    snv = ptile((B, 3 * J), "rg_snv")
    s3 = snv[:].rearrange("p (j k) -> p j k", k=3)
    V.tensor_mul(s3, n3, sn[:].unsqueeze(2).broadcast_to([B, J, 3]))
    r4 = rot[:].rearrange("p (j m n) -> p j m n", m=3, n=3)
    for m in range(3):
        V.tensor_add(r4[:, :, m, m], q3[:, :, m], dd[:])
    p = ptile((B, J), "rg_p")
    V.tensor_mul(p[:], un3[:, :, 0], n3[:, :, 1])
    V.tensor_sub(r4[:, :, 0, 1], p[:], s3[:, :, 2])
    V.tensor_add(r4[:, :, 1, 0], p[:], s3[:, :, 2])
    V.tensor_mul(p[:], un3[:, :, 0], n3[:, :, 2])
    V.tensor_add(r4[:, :, 0, 2], p[:], s3[:, :, 1])
    V.tensor_sub(r4[:, :, 2, 0], p[:], s3[:, :, 1])
    V.tensor_mul(p[:], un3[:, :, 1], n3[:, :, 2])
    V.tensor_sub(r4[:, :, 1, 2], p[:], s3[:, :, 0])
    V.tensor_add(r4[:, :, 2, 1], p[:], s3[:, :, 0])


# ================================================================ entry

_CACHED = {}
DEBUG = False


def _get_nc():
    if "nc" not in _CACHED:
        _CACHED["nc"] = _build_nc()
    return _CACHED["nc"]


PROFILE = False


def kernel(**inputs):
    in_maps, vid_all = _host_prep(inputs)
    nc = _get_nc()
    res = run_bass_kernel_spmd(nc, in_maps, core_ids=list(range(NCORES)),
                               trace=PROFILE)
    _CACHED["last_res"] = res
    out = np.zeros((B, VS, 3), np.float32)
    for c in range(NCORES):
        o = np.asarray(res.results[c]["out"]).reshape(ROWS, 3, B)
        vok = vid_all[c] >= 0
        out[:, vid_all[c][vok], :] = o[vok].transpose(2, 0, 1)
    return out

